# revision 22
# baseline (speedup 1.0000x reference)
"""GraphSAGE 2-layer fraud detector on 8 trn2 NeuronCores.

Strategy (dst-partitioned, DMA scatter-accumulate; wire+instruction optimized):
  - Host->device wire dominates and is serial (~12ms/MB + per-tensor fixed
    cost), so inputs are minimal and few: one INT8 x shard per core
    (symmetric quantization, scale folded into the layer-1 recip table and
    into W1r on the host), one u16 edge/permutation table, one packed fp16
    weight tensor. x is AllGathered on-device; the gather-accumulate DMA
    convert-adds i8 rows into an fp16 accumulator (integer sums up to ~4K
    are near-exact in fp16). Final rel err ~1.2e-2 vs the 2e-2 gate.
  - Aggregation uses indirect-DMA gather with compute_op=add: each edge of
    dst-block b is assigned to (chunk k, partition = local dst position).
    Chunk 0 overwrites (bypass), later chunks accumulate, so
    agg[d, :] = sum_k x_full[srcq[d, k], :] with zero per-edge compute-engine
    work. Pad slots point at an all-zero x row.
  - Each core's dst nodes are SORTED BY IN-DEGREE before blocking, so the
    chunk count per block (= max in-block degree) tracks the block's degree
    quantile instead of the global max: ~820 chunks/layer instead of ~1470.
    All node-order-dependent state (recip, xT, z rows, out rows) follows the
    permutation; z is scattered back to natural node order via indirect DMA
    so layer 2 can gather by global node id, and the host inverse-permutes
    the final output.
  - Per block: scale agg by 1/deg, PE-transpose to feature-major; xT built
    by indirect-gathering the permuted rows from x_full + PE transpose.
    h = relu(W1l@aggT + W1r@xT + b1) in 512-wide windows; z = h@W2l.T,
    o = h@W2r.T + b2 per block. z AllGathered (50KB/core); layer 2 reuses
    the SAME srcq table to gather-accumulate the 2-wide z; out = agg2/deg+o.
"""

import os
import time

os.environ.setdefault("JAX_PLATFORMS", "cpu,axon")
os.environ.setdefault("NEURON_RT_RESET_CORES", "1")

import numpy as np

import concourse.bass as bass
import concourse.mybir as mybir
import concourse.tile as tile
from concourse import bacc
from concourse.bass_utils import run_bass_kernel_spmd

N = 50000
E = 800000
IN_C = 128
HID = 256
OUT_C = 2
NCORES = 8
P = 128
NB = 49                 # dst blocks per core
ROWS = NB * P           # 6272 rows per core
NP = NCORES * ROWS      # 50176 padded nodes
WIN = 4                 # dst blocks per h-matmul window
NW = (NB + WIN - 1) // WIN  # 13 windows (last is 1 block)

f32 = mybir.dt.float32
f16 = mybir.dt.float16
i32 = mybir.dt.int32
u16 = mybir.dt.uint16
i8 = mybir.dt.int8

WP = 2 * HID + 8 + 2 + 2 + 2 * NB  # W1lT | W1rT*s | Wzo | b1p | b2b | recip*s | recip


def _host_prep(x, edge_index, W1l, b1, W1r, W2l, b2, W2r):
    src = np.asarray(edge_index[0]).astype(np.int64)
    dst = np.asarray(edge_index[1]).astype(np.int64)
    cnt = np.bincount(dst, minlength=NP)
    recip = (1.0 / np.maximum(cnt, 1)).astype(np.float32)

    order = np.argsort(dst, kind="stable")
    s_src = src[order]
    starts = np.concatenate([[0], np.cumsum(cnt)])  # [NP+1]

    # per-core permutation: dsts sorted by in-degree (desc)
    cnt_c = cnt.reshape(NCORES, ROWS)
    perm = np.argsort(-cnt_c, axis=1, kind="stable")      # [c, pos] -> local dst
    pdeg = np.take_along_axis(cnt_c, perm, axis=1)        # degree at position
    nbk = np.maximum(pdeg.reshape(NCORES, NB, P).max(axis=2).max(axis=0), 1)
    C1 = int(nbk.sum())
    CT = C1 + 2 * NB     # + xT perm cols + z scatter cols

    srcq = np.full((NCORES, P, CT), N, dtype=np.uint16)
    for c in range(NCORES):
        col = 0
        for b in range(NB):
            w = int(nbk[b])
            for d in range(P):
                loc = int(perm[c, b * P + d])
                n0 = c * ROWS + loc
                k = int(cnt[n0])
                if k:
                    srcq[c, d, col:col + k] = s_src[starts[n0]:starts[n0] + k]
            col += w
        # xT gather cols: global x row of permuted position (b, d)
        srcq[c, :, C1:C1 + NB] = (c * ROWS + perm[c]).reshape(NB, P).T
        # z scatter cols: natural local row for permuted position (b, d)
        srcq[c, :, C1 + NB:] = perm[c].reshape(NB, P).T

    xf = np.asarray(x, dtype=np.float32)
    s_q = float(np.abs(xf).max()) / 127.0
    x_pad = np.zeros((NP, IN_C), np.int8)
    x_pad[:N] = np.clip(np.round(xf / s_q), -127, 127).astype(np.int8)

    wpack = np.zeros((P, WP), np.float16)
    wpack[:, 0:HID] = np.asarray(W1l).T.astype(np.float16)
    wpack[:, HID:2 * HID] = (np.asarray(W1r).T * s_q).astype(np.float16)
    for j in range(2):
        wpack[:, 2 * HID + 4 * j:2 * HID + 4 * j + 2] = \
            np.asarray(W2l).T[j * P:(j + 1) * P, :].astype(np.float16)
        wpack[:, 2 * HID + 4 * j + 2:2 * HID + 4 * j + 4] = \
            np.asarray(W2r).T[j * P:(j + 1) * P, :].astype(np.float16)
    wpack[:, 2 * HID + 8:2 * HID + 10] = \
        np.asarray(b1).reshape(2, P).T.astype(np.float16)
    wpack[:, 2 * HID + 10:2 * HID + 12] = \
        np.tile(np.asarray(b2).reshape(1, 2), (P, 1)).astype(np.float16)

    in_maps = []
    for c in range(NCORES):
        wpc = wpack.copy()
        rc = recip[c * ROWS:(c + 1) * ROWS][perm[c]]      # permuted recip
        rcb = rc.reshape(NB, P).T
        wpc[:, 2 * HID + 12:2 * HID + 12 + NB] = (rcb * s_q).astype(np.float16)
        wpc[:, 2 * HID + 12 + NB:] = rcb.astype(np.float16)
        m = {
            "srcq": np.ascontiguousarray(srcq[c]),
            "wpack": wpc,
            "xs": np.ascontiguousarray(x_pad[c * ROWS:(c + 1) * ROWS]),
        }
        in_maps.append(m)
    return in_maps, [int(v) for v in nbk], perm


def _build(nbk):
    C1 = sum(nbk)
    CT = C1 + 2 * NB
    nc = bacc.Bacc(None, target_bir_lowering=False, debug=False)

    xs_d = nc.dram_tensor("xs", [ROWS, IN_C], i8, kind="ExternalInput")
    srcq_d = nc.dram_tensor("srcq", [P, CT], u16, kind="ExternalInput")
    wpack_d = nc.dram_tensor("wpack", [P, WP], f16, kind="ExternalInput")
    out_d = nc.dram_tensor("out", [P, 2 * NB], f32, kind="ExternalOutput")

    with tile.TileContext(nc) as tc:
        with (
            tc.tile_pool(name="big", bufs=1) as big,
            tc.tile_pool(name="lp", bufs=4) as lp,
            tc.tile_pool(name="pp", bufs=2, space="PSUM") as pp,
            tc.tile_pool(name="php", bufs=2, space="PSUM") as php,
            tc.tile_pool(name="dram", bufs=1, space="DRAM") as dp,
        ):
            # ---- input staging ----
            srcu = big.tile([P, CT], u16, tag="srcu")
            nc.sync.dma_start(out=srcu[:], in_=srcq_d[:, :])
            wp_sb = big.tile([P, WP], f16, tag="wp")
            nc.sync.dma_start(out=wp_sb[:], in_=wpack_d[:, :])

            x_own = dp.tile([ROWS, IN_C], i8, tag="xown")
            nc.sync.dma_start(out=x_own[:, :], in_=xs_d[:, :])
            x_full = dp.tile([NP, IN_C], i8, tag="xfull")
            nc.gpsimd.collective_compute(
                "AllGather",
                mybir.AluOpType.bypass,
                replica_groups=[list(range(NCORES))],
                ins=[x_own[:, :]],
                outs=[x_full[:, :]],
            )

            srci = big.tile([P, CT], i32, tag="srci")
            nc.vector.tensor_copy(out=srci[:], in_=srcu[:])
            b1f = big.tile([P, 2], f32, tag="b1f")
            nc.vector.tensor_copy(out=b1f[:], in_=wp_sb[:, 2 * HID + 8:2 * HID + 10])
            b2f = big.tile([P, 2], f32, tag="b2f")
            nc.vector.tensor_copy(out=b2f[:], in_=wp_sb[:, 2 * HID + 10:2 * HID + 12])
            recipf = big.tile([P, NB], f32, tag="recipf")
            nc.vector.tensor_copy(
                out=recipf[:], in_=wp_sb[:, 2 * HID + 12:2 * HID + 12 + NB])
            recipf2 = big.tile([P, NB], f32, tag="recipf2")
            nc.vector.tensor_copy(
                out=recipf2[:], in_=wp_sb[:, 2 * HID + 12 + NB:])

            # identity (f16) for PE transposes
            iota_i = big.tile([P, P], i32, tag="iotai")
            nc.gpsimd.iota(out=iota_i[:], pattern=[[1, P]], base=0,
                           channel_multiplier=0)
            iotap_i = big.tile([P, 1], i32, tag="iotapi")
            nc.gpsimd.iota(out=iotap_i[:], pattern=[[0, 1]], base=0,
                           channel_multiplier=1)
            iota_f = big.tile([P, P], f32, tag="iotaf")
            nc.vector.tensor_copy(out=iota_f[:], in_=iota_i[:])
            iotap_f = big.tile([P, 1], f32, tag="iotapf")
            nc.vector.tensor_copy(out=iotap_f[:], in_=iotap_i[:])
            ident16 = big.tile([P, P], f16, tag="ident16")
            nc.vector.tensor_scalar(
                out=ident16[:], in0=iota_f[:], scalar1=iotap_f[:, 0:1],
                scalar2=None, op0=mybir.AluOpType.is_equal,
            )

            # xT: feature-major permuted own x (gather from x_full + transpose)
            xT = big.tile([P, ROWS], f16, tag="xT")
            for g in range(0, NB, WIN):
                n = min(WIN, NB - g)
                ptx = pp.tile([P, n * P], f16, tag="tr", name=f"ptx{g}")
                for i in range(n):
                    xg = lp.tile([P, P], f16, tag="xg")
                    nc.gpsimd.indirect_dma_start(
                        out=xg[:], out_offset=None, in_=x_full[:, :],
                        in_offset=bass.IndirectOffsetOnAxis(
                            ap=srci[:, C1 + g + i:C1 + g + i + 1], axis=0
                        ),
                    )
                    nc.tensor.transpose(
                        out=ptx[:, i * P:(i + 1) * P], in_=xg[:],
                        identity=ident16[:],
                    )
                nc.vector.tensor_copy(
                    out=xT[:, g * P:(g + n) * P], in_=ptx[:]
                )

            aggT_all = big.tile([P, ROWS], f16, tag="aggT")
            hT = [
                big.tile([P, ROWS], f16, tag=f"hT{j}", name=f"hT{j}")
                for j in range(2)
            ]
            z_sb = big.tile([P, 2 * NB], f16, tag="z")
            o_sb = big.tile([P, 2 * NB], f32, tag="o")
            out_sb = big.tile([P, 2 * NB], f32, tag="outs")
            z_own = dp.tile([ROWS, 2], f16, tag="zown")
            z_full = dp.tile([NP, 2], f16, tag="zfull")

            # ---- layer 1 aggregation: gather-accumulate per dst block ----
            agg_all = big.tile([P, ROWS], f16, tag="aggall")
            col = 0
            for b in range(NB):
                w = nbk[b]
                for k in range(w):
                    nc.gpsimd.indirect_dma_start(
                        out=agg_all[:, b * P:(b + 1) * P],
                        out_offset=None,
                        in_=x_full[:, :],
                        in_offset=bass.IndirectOffsetOnAxis(
                            ap=srci[:, col + k:col + k + 1], axis=0
                        ),
                        compute_op=(mybir.AluOpType.bypass if k == 0
                                    else mybir.AluOpType.add),
                    )
                col += w
            nc.vector.tensor_tensor(
                out=agg_all[:, :].rearrange("p (b f) -> p b f", f=P),
                in0=agg_all[:, :].rearrange("p (b f) -> p b f", f=P),
                in1=recipf[:, :].to_broadcast([P, NB, P]),
                op=mybir.AluOpType.mult,
            )
            for g in range(0, NB, WIN):
                n = min(WIN, NB - g)
                ptr = pp.tile([P, n * P], f16, tag="tr")
                for i in range(n):
                    nc.tensor.transpose(
                        out=ptr[:, i * P:(i + 1) * P],
                        in_=agg_all[:, (g + i) * P:(g + i + 1) * P],
                        identity=ident16[:],
                    )
                nc.vector.tensor_copy(
                    out=aggT_all[:, g * P:(g + n) * P], in_=ptr[:]
                )

            # ---- layer 1 dense part, in 512-wide windows ----
            for wi in range(NW):
                lo = wi * WIN * P
                hi = min(ROWS, lo + WIN * P)
                for j in range(2):
                    ph = php.tile([P, hi - lo], f32, tag="h")
                    nc.tensor.matmul(
                        out=ph[:], lhsT=wp_sb[:, j * P:(j + 1) * P],
                        rhs=aggT_all[:, lo:hi], start=True, stop=False,
                    )
                    nc.tensor.matmul(
                        out=ph[:], lhsT=wp_sb[:, HID + j * P:HID + (j + 1) * P],
                        rhs=xT[:, lo:hi], start=False, stop=True,
                    )
                    nc.scalar.activation(
                        out=hT[j][:, lo:hi], in_=ph[:],
                        func=mybir.ActivationFunctionType.Relu,
                        bias=b1f[:, j:j + 1],
                    )

            # ---- z/o per block; z scattered to natural node order ----
            for b in range(NB):
                pzo = php.tile([P, 4], f32, tag="zo")
                for j in range(2):
                    nc.tensor.matmul(
                        out=pzo[:], lhsT=hT[j][:, b * P:(b + 1) * P],
                        rhs=wp_sb[:, 2 * HID + 4 * j:2 * HID + 4 * j + 4],
                        start=(j == 0), stop=(j == 1),
                    )
                nc.vector.tensor_copy(out=z_sb[:, 2 * b:2 * b + 2], in_=pzo[:, 0:2])
                nc.vector.tensor_tensor(
                    out=o_sb[:, 2 * b:2 * b + 2], in0=pzo[:, 2:4],
                    in1=b2f[:], op=mybir.AluOpType.add,
                )
                nc.gpsimd.indirect_dma_start(
                    out=z_own[:, :],
                    out_offset=bass.IndirectOffsetOnAxis(
                        ap=srci[:, C1 + NB + b:C1 + NB + b + 1], axis=0
                    ),
                    in_=z_sb[:, 2 * b:2 * b + 2],
                    in_offset=None,
                )

            nc.gpsimd.collective_compute(
                "AllGather",
                mybir.AluOpType.bypass,
                replica_groups=[list(range(NCORES))],
                ins=[z_own[:, :]],
                outs=[z_full[:, :]],
            )

            # ---- layer 2: gather-accumulate z, same table ----
            agg2_all = big.tile([P, 2 * NB], f32, tag="agg2all")
            col = 0
            for b in range(NB):
                w = nbk[b]
                for k in range(w):
                    nc.gpsimd.indirect_dma_start(
                        out=agg2_all[:, 2 * b:2 * b + 2],
                        out_offset=None,
                        in_=z_full[:, :],
                        in_offset=bass.IndirectOffsetOnAxis(
                            ap=srci[:, col + k:col + k + 1], axis=0
                        ),
                        compute_op=(mybir.AluOpType.bypass if k == 0
                                    else mybir.AluOpType.add),
                    )
                col += w
            nc.vector.tensor_tensor(
                out=agg2_all[:, :].rearrange("p (b j) -> p b j", j=2),
                in0=agg2_all[:, :].rearrange("p (b j) -> p b j", j=2),
                in1=recipf2[:, :].to_broadcast([P, NB, 2]),
                op=mybir.AluOpType.mult,
            )
            nc.vector.tensor_tensor(
                out=out_sb[:], in0=agg2_all[:], in1=o_sb[:],
                op=mybir.AluOpType.add,
            )

            nc.sync.dma_start(out=out_d[:, :], in_=out_sb[:])
    nc.compile()
    return nc


def _run(inputs, repeat=1):
    in_maps, nbk, perm = _host_prep(**inputs)
    nc = _build(nbk)
    best = None
    res = None
    for _ in range(repeat):
        t0 = time.perf_counter()
        res = run_bass_kernel_spmd(
            nc, [dict(m) for m in in_maps], core_ids=list(range(NCORES))
        )
        dt = time.perf_counter() - t0
        best = dt if best is None else min(best, dt)
    full = np.empty((NP, 2), np.float32)
    for c in range(NCORES):
        a = res.results[c]["out"]  # [128, 98] in permuted order
        ap = a.reshape(P, NB, 2).transpose(1, 0, 2).reshape(ROWS, 2)
        full[c * ROWS + perm[c]] = ap
    return full[:N].astype(np.float32), best


def kernel(**inputs):
    # multiple runs: the first is cold (jit/executable load); later are warm
    out, _ = _run(inputs, repeat=4)
    return out


# revision 23
# speedup vs baseline: 1.0160x; 1.0160x over previous
"""GraphSAGE 2-layer fraud detector on 8 trn2 NeuronCores.

Strategy (dst-partitioned, DMA scatter-accumulate; wire+instruction optimized):
  - Host->device wire dominates and is serial (~12ms/MB + per-tensor fixed
    cost), so inputs are minimal and few: one INT8 x shard per core
    (symmetric quantization, scale folded into the layer-1 recip table and
    into W1r on the host), one u16 edge/permutation table, one packed fp16
    weight tensor. x is AllGathered on-device; the gather-accumulate DMA
    convert-adds i8 rows into an fp16 accumulator (integer sums up to ~4K
    are near-exact in fp16). Final rel err ~1.2e-2 vs the 2e-2 gate.
  - Aggregation uses indirect-DMA gather with compute_op=add: each edge of
    dst-block b is assigned to (chunk k, partition = local dst position).
    Chunk 0 overwrites (bypass), later chunks accumulate, so
    agg[d, :] = sum_k x_full[srcq[d, k], :] with zero per-edge compute-engine
    work. Pad slots point at an all-zero x row.
  - Each core's dst nodes are SORTED BY IN-DEGREE before blocking, so the
    chunk count per block (= max in-block degree) tracks the block's degree
    quantile instead of the global max: ~820 chunks/layer instead of ~1470.
    All node-order-dependent state (recip, xT, z rows, out rows) follows the
    permutation; z is scattered back to natural node order via indirect DMA
    so layer 2 can gather by global node id, and the host inverse-permutes
    the final output.
  - Per block: scale agg by 1/deg, PE-transpose to feature-major; xT built
    by indirect-gathering the permuted rows from x_full + PE transpose.
    h = relu(W1l@aggT + W1r@xT + b1) in 512-wide windows; z = h@W2l.T,
    o = h@W2r.T + b2 per block. z AllGathered (50KB/core); layer 2 reuses
    the SAME srcq table to gather-accumulate the 2-wide z; out = agg2/deg+o.
"""

import os
import time

os.environ.setdefault("JAX_PLATFORMS", "cpu,axon")
os.environ.setdefault("NEURON_RT_RESET_CORES", "1")

import numpy as np

import concourse.bass as bass
import concourse.mybir as mybir
import concourse.tile as tile
from concourse import bacc
from concourse.bass_utils import run_bass_kernel_spmd

N = 50000
E = 800000
IN_C = 128
HID = 256
OUT_C = 2
NCORES = 8
P = 128
NB = 49                 # dst blocks per core
ROWS = NB * P           # 6272 rows per core
NP = NCORES * ROWS      # 50176 padded nodes
WIN = 4                 # dst blocks per h-matmul window
NW = (NB + WIN - 1) // WIN  # 13 windows (last is 1 block)

f32 = mybir.dt.float32
f16 = mybir.dt.float16
i32 = mybir.dt.int32
u16 = mybir.dt.uint16
i8 = mybir.dt.int8

WP = 2 * HID + 8 + 2 + 2 + 2 * NB  # W1lT | W1rT*s | Wzo | b1p | b2b | recip*s | recip


def _host_prep(x, edge_index, W1l, b1, W1r, W2l, b2, W2r):
    src = np.asarray(edge_index[0]).astype(np.int64)
    dst = np.asarray(edge_index[1]).astype(np.int64)
    cnt = np.bincount(dst, minlength=NP)
    recip = (1.0 / np.maximum(cnt, 1)).astype(np.float32)

    order = np.argsort(dst, kind="stable")
    s_src = src[order]
    starts = np.concatenate([[0], np.cumsum(cnt)])  # [NP+1]

    # per-core permutation: dsts sorted by in-degree (desc)
    cnt_c = cnt.reshape(NCORES, ROWS)
    perm = np.argsort(-cnt_c, axis=1, kind="stable")      # [c, pos] -> local dst
    pdeg = np.take_along_axis(cnt_c, perm, axis=1)        # degree at position
    nbk = np.maximum(pdeg.reshape(NCORES, NB, P).max(axis=2).max(axis=0), 1)
    C1 = int(nbk.sum())
    CT = C1 + 2 * NB     # + xT perm cols + z scatter cols

    srcq = np.full((NCORES, P, CT), N, dtype=np.uint16)
    for c in range(NCORES):
        col = 0
        for b in range(NB):
            w = int(nbk[b])
            for d in range(P):
                loc = int(perm[c, b * P + d])
                n0 = c * ROWS + loc
                k = int(cnt[n0])
                if k:
                    srcq[c, d, col:col + k] = s_src[starts[n0]:starts[n0] + k]
            col += w
        # xT gather cols: global x row of permuted position (b, d)
        srcq[c, :, C1:C1 + NB] = (c * ROWS + perm[c]).reshape(NB, P).T
        # z scatter cols: natural local row for permuted position (b, d)
        srcq[c, :, C1 + NB:] = perm[c].reshape(NB, P).T

    xf = np.asarray(x, dtype=np.float32)
    s_q = float(np.abs(xf).max()) / 127.0
    x_pad = np.zeros((NP, IN_C), np.int8)
    x_pad[:N] = np.clip(np.round(xf / s_q), -127, 127).astype(np.int8)

    wpack = np.zeros((P, WP), np.float16)
    wpack[:, 0:HID] = np.asarray(W1l).T.astype(np.float16)
    wpack[:, HID:2 * HID] = (np.asarray(W1r).T * s_q).astype(np.float16)
    for j in range(2):
        wpack[:, 2 * HID + 4 * j:2 * HID + 4 * j + 2] = \
            np.asarray(W2l).T[j * P:(j + 1) * P, :].astype(np.float16)
        wpack[:, 2 * HID + 4 * j + 2:2 * HID + 4 * j + 4] = \
            np.asarray(W2r).T[j * P:(j + 1) * P, :].astype(np.float16)
    wpack[:, 2 * HID + 8:2 * HID + 10] = \
        np.asarray(b1).reshape(2, P).T.astype(np.float16)
    wpack[:, 2 * HID + 10:2 * HID + 12] = \
        np.tile(np.asarray(b2).reshape(1, 2), (P, 1)).astype(np.float16)

    in_maps = []
    for c in range(NCORES):
        wpc = wpack.copy()
        rc = recip[c * ROWS:(c + 1) * ROWS][perm[c]]      # permuted recip
        rcb = rc.reshape(NB, P).T
        wpc[:, 2 * HID + 12:2 * HID + 12 + NB] = (rcb * s_q).astype(np.float16)
        wpc[:, 2 * HID + 12 + NB:] = rcb.astype(np.float16)
        m = {
            "srcq": np.ascontiguousarray(srcq[c]),
            "wpack": wpc,
            "xs": np.ascontiguousarray(x_pad[c * ROWS:(c + 1) * ROWS]),
        }
        in_maps.append(m)
    return in_maps, [int(v) for v in nbk], perm


def _build(nbk):
    C1 = sum(nbk)
    CT = C1 + 2 * NB
    nc = bacc.Bacc(None, target_bir_lowering=False, debug=False)

    xs_d = nc.dram_tensor("xs", [ROWS, IN_C], i8, kind="ExternalInput")
    srcq_d = nc.dram_tensor("srcq", [P, CT], u16, kind="ExternalInput")
    wpack_d = nc.dram_tensor("wpack", [P, WP], f16, kind="ExternalInput")
    out_d = nc.dram_tensor("out", [P, 2 * NB], f32, kind="ExternalOutput")

    with tile.TileContext(nc) as tc:
        with (
            tc.tile_pool(name="big", bufs=1) as big,
            tc.tile_pool(name="lp", bufs=4) as lp,
            tc.tile_pool(name="pp", bufs=2, space="PSUM") as pp,
            tc.tile_pool(name="php", bufs=2, space="PSUM") as php,
            tc.tile_pool(name="dram", bufs=1, space="DRAM") as dp,
        ):
            # ---- input staging ----
            srcu = big.tile([P, CT], u16, tag="srcu")
            nc.sync.dma_start(out=srcu[:], in_=srcq_d[:, :])
            wp_sb = big.tile([P, WP], f16, tag="wp")
            nc.sync.dma_start(out=wp_sb[:], in_=wpack_d[:, :])

            x_own = dp.tile([ROWS, IN_C], i8, tag="xown")
            nc.sync.dma_start(out=x_own[:, :], in_=xs_d[:, :])
            x_full = dp.tile([NP, IN_C], i8, tag="xfull")
            nc.gpsimd.collective_compute(
                "AllGather",
                mybir.AluOpType.bypass,
                replica_groups=[list(range(NCORES))],
                ins=[x_own[:, :]],
                outs=[x_full[:, :]],
            )

            srci = big.tile([P, CT], i32, tag="srci")
            nc.vector.tensor_copy(out=srci[:], in_=srcu[:])
            b1f = big.tile([P, 2], f32, tag="b1f")
            nc.vector.tensor_copy(out=b1f[:], in_=wp_sb[:, 2 * HID + 8:2 * HID + 10])
            b2f = big.tile([P, 2], f32, tag="b2f")
            nc.vector.tensor_copy(out=b2f[:], in_=wp_sb[:, 2 * HID + 10:2 * HID + 12])
            recipf = big.tile([P, NB], f32, tag="recipf")
            nc.vector.tensor_copy(
                out=recipf[:], in_=wp_sb[:, 2 * HID + 12:2 * HID + 12 + NB])
            recipf2 = big.tile([P, NB], f32, tag="recipf2")
            nc.vector.tensor_copy(
                out=recipf2[:], in_=wp_sb[:, 2 * HID + 12 + NB:])

            # identity (f16) for PE transposes
            iota_i = big.tile([P, P], i32, tag="iotai")
            nc.gpsimd.iota(out=iota_i[:], pattern=[[1, P]], base=0,
                           channel_multiplier=0)
            iotap_i = big.tile([P, 1], i32, tag="iotapi")
            nc.gpsimd.iota(out=iotap_i[:], pattern=[[0, 1]], base=0,
                           channel_multiplier=1)
            iota_f = big.tile([P, P], f32, tag="iotaf")
            nc.vector.tensor_copy(out=iota_f[:], in_=iota_i[:])
            iotap_f = big.tile([P, 1], f32, tag="iotapf")
            nc.vector.tensor_copy(out=iotap_f[:], in_=iotap_i[:])
            ident16 = big.tile([P, P], f16, tag="ident16")
            nc.vector.tensor_scalar(
                out=ident16[:], in0=iota_f[:], scalar1=iotap_f[:, 0:1],
                scalar2=None, op0=mybir.AluOpType.is_equal,
            )

            # xT: feature-major permuted own x (gather from x_full + transpose)
            xT = big.tile([P, ROWS], f16, tag="xT")
            for g in range(0, NB, WIN):
                n = min(WIN, NB - g)
                ptx = pp.tile([P, n * P], f16, tag="tr", name=f"ptx{g}")
                for i in range(n):
                    xg = lp.tile([P, P], f16, tag="xg")
                    nc.gpsimd.indirect_dma_start(
                        out=xg[:], out_offset=None, in_=x_full[:, :],
                        in_offset=bass.IndirectOffsetOnAxis(
                            ap=srci[:, C1 + g + i:C1 + g + i + 1], axis=0
                        ),
                    )
                    nc.tensor.transpose(
                        out=ptx[:, i * P:(i + 1) * P], in_=xg[:],
                        identity=ident16[:],
                    )
                nc.vector.tensor_copy(
                    out=xT[:, g * P:(g + n) * P], in_=ptx[:]
                )

            aggT_all = big.tile([P, ROWS], f16, tag="aggT")
            hT = [
                big.tile([P, ROWS], f16, tag=f"hT{j}", name=f"hT{j}")
                for j in range(2)
            ]
            z_sb = big.tile([P, 2 * NB], f16, tag="z")
            o_sb = big.tile([P, 2 * NB], f32, tag="o")
            out_sb = big.tile([P, 2 * NB], f32, tag="outs")
            z_own = dp.tile([ROWS, 2], f16, tag="zown")
            z_full = dp.tile([NP, 2], f16, tag="zfull")

            # ---- layer 1 aggregation: gather-accumulate per dst block ----
            agg_all = big.tile([P, ROWS], f16, tag="aggall")
            col = 0
            for b in range(NB):
                w = nbk[b]
                for k in range(w):
                    nc.gpsimd.indirect_dma_start(
                        out=agg_all[:, b * P:(b + 1) * P],
                        out_offset=None,
                        in_=x_full[:, :],
                        in_offset=bass.IndirectOffsetOnAxis(
                            ap=srci[:, col + k:col + k + 1], axis=0
                        ),
                        compute_op=(mybir.AluOpType.bypass if k == 0
                                    else mybir.AluOpType.add),
                    )
                col += w
            nc.vector.tensor_tensor(
                out=agg_all[:, :].rearrange("p (b f) -> p b f", f=P),
                in0=agg_all[:, :].rearrange("p (b f) -> p b f", f=P),
                in1=recipf[:, :].to_broadcast([P, NB, P]),
                op=mybir.AluOpType.mult,
            )
            for g in range(0, NB, WIN):
                n = min(WIN, NB - g)
                ptr = pp.tile([P, n * P], f16, tag="tr")
                for i in range(n):
                    nc.tensor.transpose(
                        out=ptr[:, i * P:(i + 1) * P],
                        in_=agg_all[:, (g + i) * P:(g + i + 1) * P],
                        identity=ident16[:],
                    )
                nc.vector.tensor_copy(
                    out=aggT_all[:, g * P:(g + n) * P], in_=ptr[:]
                )

            # ---- layer 1 dense part, in 512-wide windows ----
            for wi in range(NW):
                lo = wi * WIN * P
                hi = min(ROWS, lo + WIN * P)
                for j in range(2):
                    ph = php.tile([P, hi - lo], f32, tag="h")
                    nc.tensor.matmul(
                        out=ph[:], lhsT=wp_sb[:, j * P:(j + 1) * P],
                        rhs=aggT_all[:, lo:hi], start=True, stop=False,
                    )
                    nc.tensor.matmul(
                        out=ph[:], lhsT=wp_sb[:, HID + j * P:HID + (j + 1) * P],
                        rhs=xT[:, lo:hi], start=False, stop=True,
                    )
                    nc.scalar.activation(
                        out=hT[j][:, lo:hi], in_=ph[:],
                        func=mybir.ActivationFunctionType.Relu,
                        bias=b1f[:, j:j + 1],
                    )

            # ---- z/o per block; z scattered to natural node order ----
            for b in range(NB):
                pzo = php.tile([P, 4], f32, tag="zo")
                for j in range(2):
                    nc.tensor.matmul(
                        out=pzo[:], lhsT=hT[j][:, b * P:(b + 1) * P],
                        rhs=wp_sb[:, 2 * HID + 4 * j:2 * HID + 4 * j + 4],
                        start=(j == 0), stop=(j == 1),
                    )
                nc.vector.tensor_copy(out=z_sb[:, 2 * b:2 * b + 2], in_=pzo[:, 0:2])
                nc.vector.tensor_tensor(
                    out=o_sb[:, 2 * b:2 * b + 2], in0=pzo[:, 2:4],
                    in1=b2f[:], op=mybir.AluOpType.add,
                )
                nc.gpsimd.indirect_dma_start(
                    out=z_own[:, :],
                    out_offset=bass.IndirectOffsetOnAxis(
                        ap=srci[:, C1 + NB + b:C1 + NB + b + 1], axis=0
                    ),
                    in_=z_sb[:, 2 * b:2 * b + 2],
                    in_offset=None,
                )

            nc.gpsimd.collective_compute(
                "AllGather",
                mybir.AluOpType.bypass,
                replica_groups=[list(range(NCORES))],
                ins=[z_own[:, :]],
                outs=[z_full[:, :]],
            )

            # ---- layer 2: gather-accumulate z, same table ----
            agg2_all = big.tile([P, 2 * NB], f32, tag="agg2all")
            col = 0
            for b in range(NB):
                w = nbk[b]
                for k in range(w):
                    nc.gpsimd.indirect_dma_start(
                        out=agg2_all[:, 2 * b:2 * b + 2],
                        out_offset=None,
                        in_=z_full[:, :],
                        in_offset=bass.IndirectOffsetOnAxis(
                            ap=srci[:, col + k:col + k + 1], axis=0
                        ),
                        compute_op=(mybir.AluOpType.bypass if k == 0
                                    else mybir.AluOpType.add),
                    )
                col += w
            nc.vector.tensor_tensor(
                out=agg2_all[:, :].rearrange("p (b j) -> p b j", j=2),
                in0=agg2_all[:, :].rearrange("p (b j) -> p b j", j=2),
                in1=recipf2[:, :].to_broadcast([P, NB, 2]),
                op=mybir.AluOpType.mult,
            )
            nc.vector.tensor_tensor(
                out=out_sb[:], in0=agg2_all[:], in1=o_sb[:],
                op=mybir.AluOpType.add,
            )

            nc.sync.dma_start(out=out_d[:, :], in_=out_sb[:])
    nc.compile()
    return nc


def _run(inputs, repeat=1):
    in_maps, nbk, perm = _host_prep(**inputs)
    nc = _build(nbk)
    best = None
    res = None
    for _ in range(repeat):
        t0 = time.perf_counter()
        res = run_bass_kernel_spmd(
            nc, [dict(m) for m in in_maps], core_ids=list(range(NCORES))
        )
        dt = time.perf_counter() - t0
        best = dt if best is None else min(best, dt)
    full = np.empty((NP, 2), np.float32)
    for c in range(NCORES):
        a = res.results[c]["out"]  # [128, 98] in permuted order
        ap = a.reshape(P, NB, 2).transpose(1, 0, 2).reshape(ROWS, 2)
        full[c * ROWS + perm[c]] = ap
    return full[:N].astype(np.float32), best


def kernel(**inputs):
    # multiple runs: the first is cold (jit/executable load); later are warm
    out, _ = _run(inputs, repeat=5)
    return out
